# revision 1
# baseline (speedup 1.0000x reference)
"""GridGenerator_Plus kernel: batch-data-parallel across 8 NeuronCores.

Stage 1 (device, 8-way batch-sharded): cross-attention transformer -> control
points C (B,64,2).  Host: the batch-reduced pairwise-norm (needs the full
batch - this is the "all-reduce" of the squared-distance sum), then the
(N+3)x(N+3) bordered TPS solves in f32 (LAPACK; numerically equivalent to the
reference's inv(A) @ Cp).  Stage 2 (device, 8-way batch-sharded): the RBF
lifting P_hat and the final P_hat @ T.
"""
import numpy as np

B, L, D = 256, 1024, 64
H, DK = 4, 16
PY, PX = 4, 16
N = PY * PX
RH, RW = 32, 100
NGRID = RH * RW
EPS = 1e-6
NCORES = 8
BS = B // NCORES


def _build_C_np():
    gx, gy = np.meshgrid(np.linspace(-1.0, 1.0, PX, dtype=np.float64),
                         np.linspace(-1.0, 1.0, PY, dtype=np.float64), indexing='ij')
    return np.stack([gx, gy], axis=2).reshape(-1, 2).astype(np.float32)


def _build_P_np():
    gx = (np.arange(-RW, RW, 2, dtype=np.float64) + 1.0) / RW
    gy = (np.arange(-RH, RH, 2, dtype=np.float64) + 1.0) / RH
    mx, my = np.meshgrid(gx, gy, indexing='ij')
    return np.stack([mx, my], axis=2).reshape(-1, 2).astype(np.float32)


def _transformer_shard_np(Cf, W):
    """Pure-numpy fallback: control points for one batch shard (f32)."""
    g = W
    kv = Cf @ g['W_in'] + g['b_in']
    q = _build_C_np() @ g['W_emb'] + g['b_emb']                      # (N,D)
    qp = (q @ g['Wq'] + g['bq']).reshape(N, H, DK)
    kp = (kv @ g['Wk'] + g['bk']).reshape(-1, L, H, DK)
    vp = (kv @ g['Wv'] + g['bv']).reshape(-1, L, H, DK)
    sc = np.einsum('nhd,blhd->bhnl', qp, kp) / np.float32(np.sqrt(DK))
    sc = sc - sc.max(-1, keepdims=True)
    e = np.exp(sc)
    att = e / e.sum(-1, keepdims=True)
    o = np.einsum('bhnl,blhd->bnhd', att, vp).reshape(-1, N, D) @ g['Wo'] + g['bo']

    def ln(x, gg, bb):
        m = x.mean(-1, keepdims=True)
        v = ((x - m) ** 2).mean(-1, keepdims=True)
        return (x - m) / np.sqrt(v + np.float32(1e-5)) * gg + bb

    x = ln(q[None] + o, g['ln1_g'], g['ln1_b'])
    x = ln(x + np.maximum(x @ g['W1'] + g['b1'], 0) @ g['W2'] + g['b2'],
           g['ln2_g'], g['ln2_b'])
    return x @ g['W_down'] + g['b_down']


def _phat_y_shard_np(C, T):
    P = _build_P_np()
    diff = P[None, :, None, :] - C[:, None, :, :]
    rn = np.sqrt(np.maximum((diff * diff).sum(3), np.float32(1e-20)))
    rbf = rn * rn * np.log(rn + np.float32(EPS))
    Pb = np.broadcast_to(P, (C.shape[0],) + P.shape)
    P_hat = np.concatenate(
        [np.ones((C.shape[0], NGRID, 1), np.float32), Pb, rbf], axis=2)
    return (P_hat @ T).astype(np.float32)


def _solve_T(C_full, batch_C_prime):
    """Host: batch-reduced pairwise norm + bordered TPS solve, f32 like ref."""
    C = C_full.astype(np.float32)
    d = C[:, :, None, :] - C[:, None, :, :]
    sq = (d.astype(np.float64) * d.astype(np.float64)).sum((0, 3))
    eye = np.eye(N, dtype=bool)
    r = np.sqrt(np.where(eye, 1.0, sq)).astype(np.float32)
    hat = r * np.log(r)
    top = np.concatenate([np.ones((B, N, 1), np.float32), C,
                          np.broadcast_to(hat, (B, N, N))], axis=2)
    mid = np.concatenate([np.zeros((B, 2, 3), np.float32),
                          np.swapaxes(C, 1, 2)], axis=2)
    bot = np.concatenate([np.zeros((B, 1, 3), np.float32),
                          np.ones((B, 1, N), np.float32)], axis=2)
    A = np.concatenate([top, mid, bot], axis=1)
    Cp = np.concatenate([batch_C_prime.astype(np.float32),
                         np.zeros((B, 3, 2), np.float32)], axis=1)
    return np.linalg.solve(A, Cp).astype(np.float32)


def kernel(**inputs):
    inputs = {k: np.asarray(v) for k, v in inputs.items()}
    Cf_full = inputs['C_feat'].astype(np.float32)
    bcp = inputs['batch_C_prime'].astype(np.float32)
    W = {k: v.astype(np.float32) for k, v in inputs.items()
         if k not in ('C_feat', 'batch_C_prime')}

    C_full = None
    y_parts = None
    try:
        import jax
        import jax.numpy as jnp
        devs = jax.devices()[:NCORES]
        if len(devs) < NCORES:
            raise RuntimeError('not enough devices')

        qC = jnp.asarray(_build_C_np())
        P = jnp.asarray(_build_P_np())

        def stage1(Cf, g):
            kv = Cf @ g['W_in'] + g['b_in']
            q = qC @ g['W_emb'] + g['b_emb']
            qp = (q @ g['Wq'] + g['bq']).reshape(N, H, DK)
            kp = (kv @ g['Wk'] + g['bk']).reshape(BS, L, H, DK)
            vp = (kv @ g['Wv'] + g['bv']).reshape(BS, L, H, DK)
            sc = jnp.einsum('nhd,blhd->bhnl', qp, kp) / np.float32(np.sqrt(DK))
            att = jax.nn.softmax(sc, axis=-1)
            o = jnp.einsum('bhnl,blhd->bnhd', att, vp).reshape(BS, N, D) @ g['Wo'] + g['bo']

            def ln(x, gg, bb):
                m = jnp.mean(x, axis=-1, keepdims=True)
                v = jnp.mean((x - m) ** 2, axis=-1, keepdims=True)
                return (x - m) / jnp.sqrt(v + 1e-5) * gg + bb

            x = ln(q[None] + o, g['ln1_g'], g['ln1_b'])
            x = ln(x + jax.nn.relu(x @ g['W1'] + g['b1']) @ g['W2'] + g['b2'],
                   g['ln2_g'], g['ln2_b'])
            return x @ g['W_down'] + g['b_down']

        def stage2(C, T):
            diff = P[None, :, None, :] - C[:, None, :, :]
            rn = jnp.sqrt(jnp.maximum(jnp.sum(diff * diff, axis=3), 1e-20))
            rbf = rn * rn * jnp.log(rn + EPS)
            P_hat = jnp.concatenate(
                [jnp.ones((BS, NGRID, 1), jnp.float32),
                 jnp.broadcast_to(P, (BS, NGRID, 2)), rbf], axis=2)
            return P_hat @ T

        s1 = jax.jit(stage1)
        s2 = jax.jit(stage2)

        # stage 1: dispatch one shard per core, async
        gw = [{k: jax.device_put(v, d) for k, v in W.items()} for d in devs]
        cf_sh = [jax.device_put(Cf_full[i * BS:(i + 1) * BS], devs[i])
                 for i in range(NCORES)]
        c_out = [s1(cf_sh[i], gw[i]) for i in range(NCORES)]
        C_full = np.concatenate([np.asarray(c) for c in c_out], axis=0)

        # host: all-reduce'd pairwise norm + bordered solves
        T = _solve_T(C_full, bcp)

        # stage 2: rbf + final matmul per shard
        t_sh = [jax.device_put(T[i * BS:(i + 1) * BS], devs[i])
                for i in range(NCORES)]
        c_sh = [jax.device_put(C_full[i * BS:(i + 1) * BS], devs[i])
                for i in range(NCORES)]
        y_out = [s2(c_sh[i], t_sh[i]) for i in range(NCORES)]
        y_parts = [np.asarray(y) for y in y_out]
    except Exception:
        # robust fallback: full computation on host
        if C_full is None:
            C_full = np.concatenate(
                [_transformer_shard_np(Cf_full[i * BS:(i + 1) * BS], W)
                 for i in range(NCORES)], axis=0)
        T = _solve_T(C_full, bcp)
        y_parts = [_phat_y_shard_np(C_full[i * BS:(i + 1) * BS],
                                    T[i * BS:(i + 1) * BS])
                   for i in range(NCORES)]

    return np.concatenate(y_parts, axis=0).astype(np.float32)


if __name__ == '__main__':
    import time
    rng = np.random.default_rng(0)
    fake = {
        'batch_C_prime': rng.standard_normal((B, N, 2)).astype(np.float32) * 0.5,
        'C_feat': rng.standard_normal((B, L, D)).astype(np.float32),
    }
    for k, shape in [('W_in', (D, D)), ('W_emb', (2, D)), ('W_down', (D, 2)),
                     ('Wq', (D, D)), ('Wk', (D, D)), ('Wv', (D, D)), ('Wo', (D, D)),
                     ('W1', (D, D)), ('W2', (D, D))]:
        fake[k] = (rng.standard_normal(shape) / np.sqrt(shape[0])).astype(np.float32)
    for k, n in [('b_in', D), ('b_emb', D), ('b_down', 2), ('bq', D), ('bk', D),
                 ('bv', D), ('bo', D), ('b1', D), ('b2', D), ('ln1_b', D), ('ln2_b', D)]:
        fake[k] = np.zeros(n, np.float32)
    fake['ln1_g'] = np.ones(D, np.float32)
    fake['ln2_g'] = np.ones(D, np.float32)
    t0 = time.time()
    y = kernel(**fake)
    print('kernel ran in %.2fs, out shape %s' % (time.time() - t0, y.shape))

